# revision 19
# baseline (speedup 1.0000x reference)
"""PointPillarScatter kernel for 8 Trainium2 NeuronCores.

Strategy (data-parallel over batch, one core per batch element):
  host: fold BN into the 64x64 linear. Per batch: drop invalid pillars, sort
        by spatial index, and lay pillars out in a *chunk-padded* order:
        64 rows per 256-position output chunk (row 64*c + j = j-th pillar of
        chunk c, zero-padded). Features are uploaded transposed as bf16
        hi/lo pairs plus a ones row (bias via matmul).
  device, per chunk pair t (tile = 128 rows = chunks 2t, 2t+1):
    1. MLP: psum = xhi_t.T@Whi + xhi_t.T@Wlo + xlo_t.T@Whi   (PE, bf16)
    2. h tile: hi = bf16(relu(psum)) (ACT), lo = relu(psum)-hi (DVE)
    3. S one-hot: S[k, n] = (iota[n] == pos[k])  (GPSIMD is_equal, bf16)
    4. scatter matmul: out[64ch, 256pos] = hi.T@S + lo.T@S accumulated in
       PSUM f32 - scatter + transpose + zero-fill in one PE op; the two
       chunks run in opposite PE array quadrants concurrently.
    5. PSUM -> SBUF group buffer (DVE/ACT alternating), large grouped DMAs
       to the [64, ny*nx] output plane.

The program structure is input-independent; all data-dependent placement
lives in the input tensors (xhi/xlo layout, pos).
"""

import sys
import numpy as np

sys.path.insert(0, "/opt/trn_rl_repo")

import jax  # noqa: E402
from jax.sharding import Mesh, PartitionSpec  # noqa: E402
from jax.experimental.shard_map import shard_map  # noqa: E402

import concourse.bass as bass  # noqa: E402
import concourse.bacc as bacc  # noqa: E402
import concourse.mybir as mybir  # noqa: E402
from concourse import tile  # noqa: E402
from concourse import bass2jax  # noqa: E402

import ml_dtypes  # noqa: E402

F32 = mybir.dt.float32
BF16 = mybir.dt.bfloat16
NP_BF16 = ml_dtypes.bfloat16

B = 8
P_PER = 16384
C = 64
NX = 432
NY = 496
NSLOT = NX * NY          # 214272
NCH = 256                # output positions per chunk
NCHUNK = NSLOT // NCH    # 837
NPAIR = (NCHUNK + 1) // 2  # 419 (last pair has a dummy upper chunk)
RPC = 64                 # padded pillar rows per chunk
NROW = NPAIR * 128       # padded pillar rows total (53632)
GRP = 11                 # chunk pairs per output DMA group (418 = 38*11)
SEGP = 16                # chunk pairs per streamed x segment
EPS = 1e-5

_cache = {}


def _build_program(reps=1):
    nc = bacc.Bacc(None, target_bir_lowering=False, debug=False)

    xhi = nc.dram_tensor("xhi", [C + 1, NROW], BF16, kind="ExternalInput")
    xlo = nc.dram_tensor("xlo", [C + 1, NROW], BF16, kind="ExternalInput")
    w = nc.dram_tensor("w", [C + 1, 2 * C], BF16, kind="ExternalInput")
    pos = nc.dram_tensor("pos", [128, NPAIR], F32, kind="ExternalInput")
    out = nc.dram_tensor("out", [C, NSLOT], F32, kind="ExternalOutput")

    with tile.TileContext(nc) as tc:
        with (
            tc.tile_pool(name="const", bufs=1) as cpool,
            tc.tile_pool(name="xin", bufs=3) as xpool,
            tc.tile_pool(name="mlp_ps", bufs=3, space="PSUM") as mlp_ps,
            tc.tile_pool(name="scat_ps", bufs=5, space="PSUM") as scat_ps,
            tc.tile_pool(name="hpool", bufs=6) as hpool,
            tc.tile_pool(name="spool", bufs=6) as spool,
            tc.tile_pool(name="opool", bufs=2) as opool,
        ):
            # ---- constants / inputs to SBUF
            w_sb = cpool.tile([C + 1, 2 * C], BF16)
            nc.sync.dma_start(w_sb[:], w.ap())
            pos_sb = cpool.tile([128, NPAIR], F32)
            nc.sync.dma_start(pos_sb[:], pos.ap())
            iota_sb = cpool.tile([128, NCH], BF16)
            nc.gpsimd.iota(
                iota_sb[:], pattern=[[1, NCH]], base=0, channel_multiplier=0,
                allow_small_or_imprecise_dtypes=True,
            )

            def _emit_body():
                _emit_pairs(nc, tc, xhi, xlo, xpool, w_sb, pos_sb, iota_sb,
                            out, mlp_ps, scat_ps, hpool, spool, opool)

            if reps == 1:
                _emit_body()
            else:
                with tc.For_i(0, reps, 1):
                    _emit_body()

    nc.compile()
    return nc


def _emit_pairs(nc, tc, xhi, xlo, xpool, w_sb, pos_sb, iota_sb, out,
                mlp_ps, scat_ps, hpool, spool, opool):
    whi = w_sb[:, 0:C]
    wlo = w_sb[:, C:2 * C]
    ob = None
    xhi_seg = xlo_seg = None
    for t in range(NPAIR):
        if t % SEGP == 0:
            npr = min(SEGP, NPAIR - t)
            xhi_seg = xpool.tile([C + 1, 128 * SEGP], BF16)
            nc.sync.dma_start(
                xhi_seg[:, 0:128 * npr],
                xhi.ap()[:, t * 128:(t + npr) * 128],
            )
            xlo_seg = xpool.tile([C + 1, 128 * SEGP], BF16)
            nc.sync.dma_start(
                xlo_seg[:, 0:128 * npr],
                xlo.ap()[:, t * 128:(t + npr) * 128],
            )
        # ---- MLP for this tile's 128 padded pillar rows
        mp = mlp_ps.tile([128, C], F32)
        xh = xhi_seg[:, bass.ts(t % SEGP, 128)]
        xl = xlo_seg[:, bass.ts(t % SEGP, 128)]
        nc.tensor.matmul(mp[:], lhsT=xh, rhs=whi, start=True, stop=False)
        nc.tensor.matmul(mp[:], lhsT=xh, rhs=wlo, start=False, stop=False)
        nc.tensor.matmul(mp[:], lhsT=xl, rhs=whi, start=False, stop=True)

        ht = hpool.tile([128, 2 * C], BF16)
        hi = ht[:, 0:C]
        lo = ht[:, C:2 * C]
        nc.scalar.activation(hi, mp[:], mybir.ActivationFunctionType.Relu)
        nc.vector.scalar_tensor_tensor(
            lo, mp[:], 0.0, hi,
            op0=mybir.AluOpType.max, op1=mybir.AluOpType.subtract,
        )

        # ---- one-hot position matrix
        st = spool.tile([128, NCH], BF16)
        nc.gpsimd.tensor_scalar(
            st[:], iota_sb[:], pos_sb[:, t:t + 1], None,
            op0=mybir.AluOpType.is_equal,
        )

        # ---- scatter matmuls (chunk 2t in rows/cols 0-63, 2t+1 in 64-127)
        ps = scat_ps.tile([128, NCH], F32)
        nc.tensor.matmul(
            ps[0:64, :], lhsT=ht[0:64, 0:C], rhs=st[0:64, :],
            start=True, stop=False, tile_position=(0, 0),
        )
        nc.tensor.matmul(
            ps[0:64, :], lhsT=ht[0:64, C:2 * C], rhs=st[0:64, :],
            start=False, stop=True, tile_position=(0, 0),
        )
        nc.tensor.matmul(
            ps[64:128, :], lhsT=ht[64:128, 0:C], rhs=st[64:128, :],
            start=True, stop=False, tile_position=(64, 64),
        )
        nc.tensor.matmul(
            ps[64:128, :], lhsT=ht[64:128, C:2 * C], rhs=st[64:128, :],
            start=False, stop=True, tile_position=(64, 64),
        )

        # ---- PSUM -> group buffer (alternate engines), grouped DMA out
        j = t % GRP
        if j == 0:
            ob = opool.tile([128, GRP * NCH], F32)
        if t % 2 == 0:
            nc.vector.tensor_copy(ob[:, bass.ts(j, NCH)], ps[:])
        else:
            nc.scalar.activation(
                ob[:, bass.ts(j, NCH)], ps[:],
                mybir.ActivationFunctionType.Copy,
            )

        if t == NPAIR - 1:
            # final pair: only the lower half (chunk 836) exists
            dst = out.ap()[:, NCHUNK * NCH - NCH:]
            nc.sync.dma_start(dst, ob[0:64, 0:NCH])
        elif j == GRP - 1:
            g = t // GRP
            span = out.ap()[:, g * GRP * 2 * NCH:(g + 1) * GRP * 2 * NCH]
            span = span.rearrange("c (j x) -> c j x", j=GRP)
            nc.sync.dma_start(span[:, :, 0:NCH], ob[0:64, :])
            nc.sync.dma_start(span[:, :, NCH:2 * NCH], ob[64:128, :])


class _Runner:
    """Compile-once executor for the SPMD program on 8 cores."""

    def __init__(self, nc):
        self.nc = nc
        bass2jax.install_neuronx_cc_hook()
        part_name = (nc.partition_id_tensor.name
                     if nc.partition_id_tensor else None)
        in_names, out_names, out_avals = [], [], []
        for alloc in nc.m.functions[0].allocations:
            if not isinstance(alloc, mybir.MemoryLocationSet):
                continue
            name = alloc.memorylocations[0].name
            if alloc.kind == "ExternalInput":
                if name != part_name:
                    in_names.append(name)
            elif alloc.kind == "ExternalOutput":
                out_names.append(name)
                out_avals.append(jax.core.ShapedArray(
                    tuple(alloc.tensor_shape), mybir.dt.np(alloc.dtype)))
        self.in_names = in_names
        self.out_names = out_names
        self.out_avals = out_avals
        n_io = len(in_names) + len(out_names)

        devices = jax.devices()[:B]
        self.mesh = Mesh(np.asarray(devices), ("core",))
        all_in_names = list(in_names) + list(out_names)
        if part_name is not None:
            all_in_names.append(part_name)
        all_in_names = tuple(all_in_names)

        def _body(*args):
            operands = list(args)
            if part_name is not None:
                operands.append(bass2jax.partition_id_tensor())
            outs = bass2jax._bass_exec_p.bind(
                *operands,
                out_avals=tuple(out_avals),
                in_names=all_in_names,
                out_names=tuple(out_names),
                lowering_input_output_aliases=(),
                sim_require_finite=True,
                sim_require_nnan=True,
                nc=nc,
            )
            return tuple(outs)

        self.fn = jax.jit(
            shard_map(
                _body, mesh=self.mesh,
                in_specs=(PartitionSpec("core"),) * n_io,
                out_specs=(PartitionSpec("core"),) * len(out_names),
                check_rep=False,
            ),
            keep_unused=True,
        )
        # persistent pre-zeroed "output seed" buffers (kernel writes every
        # element, so their contents are never observed). Created on-device
        # to avoid shipping hundreds of MB through the axon tunnel.
        from jax.sharding import NamedSharding
        import jax.numpy as jnp
        sh = NamedSharding(self.mesh, PartitionSpec("core"))
        self.zero_outs = [
            jax.jit(
                lambda a=a: jnp.zeros((B * a.shape[0], *a.shape[1:]), a.dtype),
                out_shardings=sh,
            )()
            for a in out_avals
        ]

    def to_device(self, arr):
        from jax.sharding import NamedSharding
        return jax.device_put(arr, NamedSharding(self.mesh, PartitionSpec("core")))

    def concat_inputs(self, in_maps):
        return [
            np.concatenate([m[name] for m in in_maps], axis=0)
            for name in self.in_names
        ]

    def run(self, dev_or_np_inputs):
        return self.fn(*dev_or_np_inputs, *self.zero_outs)

    def run_maps(self, in_maps, fetch=True):
        outs = self.run(self.concat_inputs(in_maps))
        if not fetch:
            jax.block_until_ready(outs)
            return None
        res = []
        for c in range(B):
            res.append({
                name: np.asarray(outs[i]).reshape(
                    B, *self.out_avals[i].shape)[c]
                for i, name in enumerate(self.out_names)
            })
        return res


def _get_runner(reps=1):
    key = f"runner{reps}"
    if key not in _cache:
        _cache[key] = _Runner(_build_program(reps))
    return _cache[key]


def _split_hi_lo(a):
    hi = a.astype(NP_BF16)
    lo = (a - hi.astype(np.float32)).astype(NP_BF16)
    return hi, lo


def _host_prep(pillar_features, voxel_coords, topk_w, topk_b, bn_gamma,
               bn_beta, bn_mean, bn_var):
    """Build per-core input maps."""
    s = (bn_gamma / np.sqrt(bn_var + EPS)).astype(np.float32)
    wf = (topk_w * s[None, :]).astype(np.float32)
    bf = ((topk_b - bn_mean) * s + bn_beta).astype(np.float32)
    w_aug = np.concatenate([wf, bf[None, :]], axis=0)  # [65, 64] f32
    whi, wlo = _split_hi_lo(w_aug)
    w_in = np.concatenate([whi, wlo], axis=1)  # [65, 128] bf16

    in_maps = []
    for b in range(B):
        sl = slice(b * P_PER, (b + 1) * P_PER)
        cb = voxel_coords[sl]
        xb = pillar_features[sl]
        valid = cb[:, 4] != -1
        g = (cb[valid, 1] + cb[valid, 2] * NX + cb[valid, 3]).astype(np.int64)
        xv = xb[valid]
        order = np.argsort(g, kind="stable")
        g = g[order]
        xv = xv[order]

        bounds = np.searchsorted(g, np.arange(NCHUNK + 1) * NCH)
        ncs = np.diff(bounds)
        if ncs.max() > RPC:
            raise RuntimeError(
                f"chunk with {ncs.max()} pillars exceeds capacity {RPC}"
            )

        # padded layout: row 64*c + j = j-th pillar of chunk c
        rows = np.arange(NCHUNK) * RPC  # start row of each chunk
        src = np.full(NROW, -1, np.int64)
        posm = np.full((128, NPAIR), -1.0, np.float32)
        for c in range(NCHUNK):
            lo_, hi_ = bounds[c], bounds[c + 1]
            n = hi_ - lo_
            if n == 0:
                continue
            r0 = rows[c]
            src[r0:r0 + n] = np.arange(lo_, hi_)
            t = c // 2
            p0 = (c % 2) * RPC
            posm[p0:p0 + n, t] = (g[lo_:hi_] - c * NCH).astype(np.float32)

        xpad = np.zeros((NROW, C + 1), np.float32)
        sel = src >= 0
        xpad[sel, :C] = xv[src[sel]]
        xpad[:, C] = 1.0
        xhi, xlo = _split_hi_lo(xpad.T)  # [65, NROW] bf16 each

        in_maps.append({
            "xhi": np.ascontiguousarray(xhi),
            "xlo": np.ascontiguousarray(xlo),
            "w": w_in,
            "pos": posm,
        })
    return in_maps


def kernel(**inputs):
    pillar_features = np.asarray(inputs["pillar_features"], np.float32)
    voxel_coords = np.asarray(inputs["voxel_coords"])
    topk_w = np.asarray(inputs["topk_w"], np.float32)
    topk_b = np.asarray(inputs["topk_b"], np.float32)
    bn_gamma = np.asarray(inputs["bn_gamma"], np.float32)
    bn_beta = np.asarray(inputs["bn_beta"], np.float32)
    bn_mean = np.asarray(inputs["bn_mean"], np.float32)
    bn_var = np.asarray(inputs["bn_var"], np.float32)
    assert int(np.asarray(inputs["batch_size"])) == B
    assert int(np.asarray(inputs["nx"])) == NX
    assert int(np.asarray(inputs["ny"])) == NY

    in_maps = _host_prep(pillar_features, voxel_coords, topk_w, topk_b,
                         bn_gamma, bn_beta, bn_mean, bn_var)
    res = _get_runner().run_maps(in_maps)
    out = np.stack([res[b]["out"] for b in range(B)], axis=0)
    return out.reshape(B, C, NY, NX)


# revision 20
# speedup vs baseline: 3.5336x; 3.5336x over previous
"""PointPillarScatter kernel for 8 Trainium2 NeuronCores.

Strategy (data-parallel over batch, one core per batch element):
  host: fold BN into the 64x64 linear. Per batch: drop invalid pillars, sort
        by spatial index, and lay pillars out in a *chunk-padded* order:
        64 rows per 256-position output chunk (row 64*c + j = j-th pillar of
        chunk c, zero-padded). Features are uploaded transposed as bf16
        hi/lo pairs plus a ones row (bias via matmul).
  device, per chunk pair t (tile = 128 rows = chunks 2t, 2t+1):
    1. MLP: psum = xhi_t.T@Whi + xhi_t.T@Wlo + xlo_t.T@Whi   (PE, bf16)
    2. h tile: hi = bf16(relu(psum)) (ACT), lo = relu(psum)-hi (DVE)
    3. S one-hot: S[k, n] = (iota[n] == pos[k])  (GPSIMD is_equal, bf16)
    4. scatter matmul: out[64ch, 256pos] = hi.T@S + lo.T@S accumulated in
       PSUM f32 - scatter + transpose + zero-fill in one PE op; the two
       chunks run in opposite PE array quadrants concurrently.
    5. PSUM -> SBUF group buffer (DVE/ACT alternating), large grouped DMAs
       to the [64, ny*nx] output plane.

The program structure is input-independent; all data-dependent placement
lives in the input tensors (xhi/xlo layout, pos).
"""

import sys
import numpy as np

sys.path.insert(0, "/opt/trn_rl_repo")

import jax  # noqa: E402
from jax.sharding import Mesh, PartitionSpec  # noqa: E402
from jax.experimental.shard_map import shard_map  # noqa: E402

import concourse.bass as bass  # noqa: E402
import concourse.bacc as bacc  # noqa: E402
import concourse.mybir as mybir  # noqa: E402
from concourse import tile  # noqa: E402
from concourse import bass2jax  # noqa: E402

import ml_dtypes  # noqa: E402

F32 = mybir.dt.float32
BF16 = mybir.dt.bfloat16
NP_BF16 = ml_dtypes.bfloat16

B = 8
P_PER = 16384
C = 64
NX = 432
NY = 496
NSLOT = NX * NY          # 214272
NCH = 256                # output positions per chunk
NCHUNK = NSLOT // NCH    # 837
NPAIR = (NCHUNK + 1) // 2  # 419 (last pair has a dummy upper chunk)
RPC = 64                 # padded pillar rows per chunk
NROW = NPAIR * 128       # padded pillar rows total (53632)
GRP = 11                 # chunk pairs per output DMA group (418 = 38*11)
SEGP = 16                # chunk pairs per streamed x segment
EPS = 1e-5

_cache = {}


def _build_program(reps=1):
    nc = bacc.Bacc(None, target_bir_lowering=False, debug=False)

    xhi = nc.dram_tensor("xhi", [C + 1, NROW], BF16, kind="ExternalInput")
    xlo = nc.dram_tensor("xlo", [C + 1, NROW], BF16, kind="ExternalInput")
    w = nc.dram_tensor("w", [C + 1, 2 * C], BF16, kind="ExternalInput")
    pos = nc.dram_tensor("pos", [128, NPAIR], F32, kind="ExternalInput")
    out = nc.dram_tensor("out", [C, NSLOT], F32, kind="ExternalOutput")

    with tile.TileContext(nc) as tc:
        with (
            tc.tile_pool(name="const", bufs=1) as cpool,
            tc.tile_pool(name="xin", bufs=3) as xpool,
            tc.tile_pool(name="mlp_ps", bufs=3, space="PSUM") as mlp_ps,
            tc.tile_pool(name="scat_ps", bufs=5, space="PSUM") as scat_ps,
            tc.tile_pool(name="hpool", bufs=6) as hpool,
            tc.tile_pool(name="spool", bufs=6) as spool,
            tc.tile_pool(name="opool", bufs=2) as opool,
        ):
            # ---- constants / inputs to SBUF
            w_sb = cpool.tile([C + 1, 2 * C], BF16)
            nc.sync.dma_start(w_sb[:], w.ap())
            pos_sb = cpool.tile([128, NPAIR], F32)
            nc.sync.dma_start(pos_sb[:], pos.ap())
            iota_sb = cpool.tile([128, NCH], BF16)
            nc.gpsimd.iota(
                iota_sb[:], pattern=[[1, NCH]], base=0, channel_multiplier=0,
                allow_small_or_imprecise_dtypes=True,
            )

            def _emit_body():
                _emit_pairs(nc, tc, xhi, xlo, xpool, w_sb, pos_sb, iota_sb,
                            out, mlp_ps, scat_ps, hpool, spool, opool)

            if reps == 1:
                _emit_body()
            else:
                with tc.For_i(0, reps, 1):
                    _emit_body()

    nc.compile()
    return nc


def _emit_pairs(nc, tc, xhi, xlo, xpool, w_sb, pos_sb, iota_sb, out,
                mlp_ps, scat_ps, hpool, spool, opool):
    whi = w_sb[:, 0:C]
    wlo = w_sb[:, C:2 * C]
    ob = None
    xhi_seg = xlo_seg = None
    for t in range(NPAIR):
        if t % SEGP == 0:
            npr = min(SEGP, NPAIR - t)
            xhi_seg = xpool.tile([C + 1, 128 * SEGP], BF16)
            nc.sync.dma_start(
                xhi_seg[:, 0:128 * npr],
                xhi.ap()[:, t * 128:(t + npr) * 128],
            )
            xlo_seg = xpool.tile([C + 1, 128 * SEGP], BF16)
            nc.sync.dma_start(
                xlo_seg[:, 0:128 * npr],
                xlo.ap()[:, t * 128:(t + npr) * 128],
            )
        # ---- MLP for this tile's 128 padded pillar rows
        mp = mlp_ps.tile([128, C], F32)
        xh = xhi_seg[:, bass.ts(t % SEGP, 128)]
        xl = xlo_seg[:, bass.ts(t % SEGP, 128)]
        nc.tensor.matmul(mp[:], lhsT=xh, rhs=whi, start=True, stop=False)
        nc.tensor.matmul(mp[:], lhsT=xh, rhs=wlo, start=False, stop=False)
        nc.tensor.matmul(mp[:], lhsT=xl, rhs=whi, start=False, stop=True)

        ht = hpool.tile([128, 2 * C], BF16)
        hi = ht[:, 0:C]
        lo = ht[:, C:2 * C]
        nc.scalar.activation(hi, mp[:], mybir.ActivationFunctionType.Relu)
        nc.vector.scalar_tensor_tensor(
            lo, mp[:], 0.0, hi,
            op0=mybir.AluOpType.max, op1=mybir.AluOpType.subtract,
        )

        # ---- one-hot position matrix
        st = spool.tile([128, NCH], BF16)
        nc.vector.tensor_scalar(
            st[:], iota_sb[:], pos_sb[:, t:t + 1], None,
            op0=mybir.AluOpType.is_equal,
        )

        # ---- scatter matmuls (chunk 2t in rows/cols 0-63, 2t+1 in 64-127)
        ps = scat_ps.tile([128, NCH], F32)
        nc.tensor.matmul(
            ps[0:64, :], lhsT=ht[0:64, 0:C], rhs=st[0:64, :],
            start=True, stop=False, tile_position=(0, 0),
        )
        nc.tensor.matmul(
            ps[0:64, :], lhsT=ht[0:64, C:2 * C], rhs=st[0:64, :],
            start=False, stop=True, tile_position=(0, 0),
        )
        nc.tensor.matmul(
            ps[64:128, :], lhsT=ht[64:128, 0:C], rhs=st[64:128, :],
            start=True, stop=False, tile_position=(64, 64),
        )
        nc.tensor.matmul(
            ps[64:128, :], lhsT=ht[64:128, C:2 * C], rhs=st[64:128, :],
            start=False, stop=True, tile_position=(64, 64),
        )

        # ---- PSUM -> group buffer (alternate engines), grouped DMA out
        j = t % GRP
        if j == 0:
            ob = opool.tile([128, GRP * NCH], F32)
        if t % 2 == 0:
            nc.vector.tensor_copy(ob[:, bass.ts(j, NCH)], ps[:])
        else:
            nc.scalar.activation(
                ob[:, bass.ts(j, NCH)], ps[:],
                mybir.ActivationFunctionType.Copy,
            )

        if t == NPAIR - 1:
            # final pair: only the lower half (chunk 836) exists
            dst = out.ap()[:, NCHUNK * NCH - NCH:]
            nc.sync.dma_start(dst, ob[0:64, 0:NCH])
        elif j == GRP - 1:
            g = t // GRP
            span = out.ap()[:, g * GRP * 2 * NCH:(g + 1) * GRP * 2 * NCH]
            span = span.rearrange("c (j x) -> c j x", j=GRP)
            nc.sync.dma_start(span[:, :, 0:NCH], ob[0:64, :])
            nc.sync.dma_start(span[:, :, NCH:2 * NCH], ob[64:128, :])


class _Runner:
    """Compile-once executor for the SPMD program on 8 cores."""

    def __init__(self, nc):
        self.nc = nc
        bass2jax.install_neuronx_cc_hook()
        part_name = (nc.partition_id_tensor.name
                     if nc.partition_id_tensor else None)
        in_names, out_names, out_avals = [], [], []
        for alloc in nc.m.functions[0].allocations:
            if not isinstance(alloc, mybir.MemoryLocationSet):
                continue
            name = alloc.memorylocations[0].name
            if alloc.kind == "ExternalInput":
                if name != part_name:
                    in_names.append(name)
            elif alloc.kind == "ExternalOutput":
                out_names.append(name)
                out_avals.append(jax.core.ShapedArray(
                    tuple(alloc.tensor_shape), mybir.dt.np(alloc.dtype)))
        self.in_names = in_names
        self.out_names = out_names
        self.out_avals = out_avals
        n_io = len(in_names) + len(out_names)

        devices = jax.devices()[:B]
        self.mesh = Mesh(np.asarray(devices), ("core",))
        all_in_names = list(in_names) + list(out_names)
        if part_name is not None:
            all_in_names.append(part_name)
        all_in_names = tuple(all_in_names)

        def _body(*args):
            operands = list(args)
            if part_name is not None:
                operands.append(bass2jax.partition_id_tensor())
            outs = bass2jax._bass_exec_p.bind(
                *operands,
                out_avals=tuple(out_avals),
                in_names=all_in_names,
                out_names=tuple(out_names),
                lowering_input_output_aliases=(),
                sim_require_finite=True,
                sim_require_nnan=True,
                nc=nc,
            )
            return tuple(outs)

        self.fn = jax.jit(
            shard_map(
                _body, mesh=self.mesh,
                in_specs=(PartitionSpec("core"),) * n_io,
                out_specs=(PartitionSpec("core"),) * len(out_names),
                check_rep=False,
            ),
            keep_unused=True,
        )
        # persistent pre-zeroed "output seed" buffers (kernel writes every
        # element, so their contents are never observed). Created on-device
        # to avoid shipping hundreds of MB through the axon tunnel.
        from jax.sharding import NamedSharding
        import jax.numpy as jnp
        sh = NamedSharding(self.mesh, PartitionSpec("core"))
        self.zero_outs = [
            jax.jit(
                lambda a=a: jnp.zeros((B * a.shape[0], *a.shape[1:]), a.dtype),
                out_shardings=sh,
            )()
            for a in out_avals
        ]

    def to_device(self, arr):
        from jax.sharding import NamedSharding
        return jax.device_put(arr, NamedSharding(self.mesh, PartitionSpec("core")))

    def concat_inputs(self, in_maps):
        return [
            np.concatenate([m[name] for m in in_maps], axis=0)
            for name in self.in_names
        ]

    def run(self, dev_or_np_inputs):
        return self.fn(*dev_or_np_inputs, *self.zero_outs)

    def run_maps(self, in_maps, fetch=True):
        outs = self.run(self.concat_inputs(in_maps))
        if not fetch:
            jax.block_until_ready(outs)
            return None
        res = []
        for c in range(B):
            res.append({
                name: np.asarray(outs[i]).reshape(
                    B, *self.out_avals[i].shape)[c]
                for i, name in enumerate(self.out_names)
            })
        return res


def _get_runner(reps=1):
    key = f"runner{reps}"
    if key not in _cache:
        _cache[key] = _Runner(_build_program(reps))
    return _cache[key]


def _split_hi_lo(a):
    hi = a.astype(NP_BF16)
    lo = (a - hi.astype(np.float32)).astype(NP_BF16)
    return hi, lo


def _host_prep(pillar_features, voxel_coords, topk_w, topk_b, bn_gamma,
               bn_beta, bn_mean, bn_var):
    """Build per-core input maps."""
    s = (bn_gamma / np.sqrt(bn_var + EPS)).astype(np.float32)
    wf = (topk_w * s[None, :]).astype(np.float32)
    bf = ((topk_b - bn_mean) * s + bn_beta).astype(np.float32)
    w_aug = np.concatenate([wf, bf[None, :]], axis=0)  # [65, 64] f32
    whi, wlo = _split_hi_lo(w_aug)
    w_in = np.concatenate([whi, wlo], axis=1)  # [65, 128] bf16

    in_maps = []
    for b in range(B):
        sl = slice(b * P_PER, (b + 1) * P_PER)
        cb = voxel_coords[sl]
        xb = pillar_features[sl]
        valid = cb[:, 4] != -1
        g = (cb[valid, 1] + cb[valid, 2] * NX + cb[valid, 3]).astype(np.int64)
        xv = xb[valid]
        order = np.argsort(g, kind="stable")
        g = g[order]
        xv = xv[order]

        bounds = np.searchsorted(g, np.arange(NCHUNK + 1) * NCH)
        ncs = np.diff(bounds)
        if ncs.max() > RPC:
            raise RuntimeError(
                f"chunk with {ncs.max()} pillars exceeds capacity {RPC}"
            )

        # padded layout: row 64*c + j = j-th pillar of chunk c
        rows = np.arange(NCHUNK) * RPC  # start row of each chunk
        src = np.full(NROW, -1, np.int64)
        posm = np.full((128, NPAIR), -1.0, np.float32)
        for c in range(NCHUNK):
            lo_, hi_ = bounds[c], bounds[c + 1]
            n = hi_ - lo_
            if n == 0:
                continue
            r0 = rows[c]
            src[r0:r0 + n] = np.arange(lo_, hi_)
            t = c // 2
            p0 = (c % 2) * RPC
            posm[p0:p0 + n, t] = (g[lo_:hi_] - c * NCH).astype(np.float32)

        xpad = np.zeros((NROW, C + 1), np.float32)
        sel = src >= 0
        xpad[sel, :C] = xv[src[sel]]
        xpad[:, C] = 1.0
        xhi, xlo = _split_hi_lo(xpad.T)  # [65, NROW] bf16 each

        in_maps.append({
            "xhi": np.ascontiguousarray(xhi),
            "xlo": np.ascontiguousarray(xlo),
            "w": w_in,
            "pos": posm,
        })
    return in_maps


def kernel(**inputs):
    pillar_features = np.asarray(inputs["pillar_features"], np.float32)
    voxel_coords = np.asarray(inputs["voxel_coords"])
    topk_w = np.asarray(inputs["topk_w"], np.float32)
    topk_b = np.asarray(inputs["topk_b"], np.float32)
    bn_gamma = np.asarray(inputs["bn_gamma"], np.float32)
    bn_beta = np.asarray(inputs["bn_beta"], np.float32)
    bn_mean = np.asarray(inputs["bn_mean"], np.float32)
    bn_var = np.asarray(inputs["bn_var"], np.float32)
    assert int(np.asarray(inputs["batch_size"])) == B
    assert int(np.asarray(inputs["nx"])) == NX
    assert int(np.asarray(inputs["ny"])) == NY

    in_maps = _host_prep(pillar_features, voxel_coords, topk_w, topk_b,
                         bn_gamma, bn_beta, bn_mean, bn_var)
    res = _get_runner().run_maps(in_maps)
    out = np.stack([res[b]["out"] for b in range(B)], axis=0)
    return out.reshape(B, C, NY, NX)


# revision 21
# speedup vs baseline: 3.5975x; 1.0181x over previous
"""PointPillarScatter kernel for 8 Trainium2 NeuronCores.

Strategy (data-parallel over batch, one core per batch element):
  host: fold BN into the 64x64 linear. Per batch: drop invalid pillars, sort
        by spatial index, and lay pillars out in a *chunk-padded* order:
        64 rows per 256-position output chunk (row 64*c + j = j-th pillar of
        chunk c, zero-padded). Features are uploaded transposed as bf16
        hi/lo pairs plus a ones row (bias via matmul).
  device, per chunk pair t (tile = 128 rows = chunks 2t, 2t+1):
    1. MLP: psum = xhi_t.T@Whi + xhi_t.T@Wlo + xlo_t.T@Whi   (PE, bf16)
    2. h tile: hi = bf16(relu(psum)) (ACT), lo = relu(psum)-hi (DVE)
    3. S one-hot: S[k, n] = (iota[n] == pos[k])  (GPSIMD is_equal, bf16)
    4. scatter matmul: out[64ch, 256pos] = hi.T@S + lo.T@S accumulated in
       PSUM f32 - scatter + transpose + zero-fill in one PE op; the two
       chunks run in opposite PE array quadrants concurrently.
    5. PSUM -> SBUF group buffer (DVE/ACT alternating), large grouped DMAs
       to the [64, ny*nx] output plane.

The program structure is input-independent; all data-dependent placement
lives in the input tensors (xhi/xlo layout, pos).
"""

import sys
import numpy as np

sys.path.insert(0, "/opt/trn_rl_repo")

import jax  # noqa: E402
from jax.sharding import Mesh, PartitionSpec  # noqa: E402
from jax.experimental.shard_map import shard_map  # noqa: E402

import concourse.bass as bass  # noqa: E402
import concourse.bacc as bacc  # noqa: E402
import concourse.mybir as mybir  # noqa: E402
from concourse import tile  # noqa: E402
from concourse import bass2jax  # noqa: E402

import ml_dtypes  # noqa: E402

F32 = mybir.dt.float32
BF16 = mybir.dt.bfloat16
NP_BF16 = ml_dtypes.bfloat16

B = 8
P_PER = 16384
C = 64
NX = 432
NY = 496
NSLOT = NX * NY          # 214272
NCH = 256                # output positions per chunk
NCHUNK = NSLOT // NCH    # 837
NPAIR = (NCHUNK + 1) // 2  # 419 (last pair has a dummy upper chunk)
RPC = 64                 # padded pillar rows per chunk
NROW = NPAIR * 128       # padded pillar rows total (53632)
GRP = 22                 # chunk pairs per output DMA group (418 = 19*22)
SEGP = 16                # chunk pairs per streamed x segment
KB = 8                   # chunk pairs per batched MLP/S-build op
EPS = 1e-5

_cache = {}


def _build_program(reps=1):
    nc = bacc.Bacc(None, target_bir_lowering=False, debug=False)

    xhi = nc.dram_tensor("xhi", [C + 1, NROW], BF16, kind="ExternalInput")
    xlo = nc.dram_tensor("xlo", [C + 1, NROW], BF16, kind="ExternalInput")
    w = nc.dram_tensor("w", [C + 1, 2 * C], BF16, kind="ExternalInput")
    pos = nc.dram_tensor("pos", [128, NPAIR], BF16, kind="ExternalInput")
    out = nc.dram_tensor("out", [C, NSLOT], F32, kind="ExternalOutput")

    with tile.TileContext(nc) as tc:
        with (
            tc.tile_pool(name="const", bufs=1) as cpool,
            tc.tile_pool(name="xin", bufs=3) as xpool,
            tc.tile_pool(name="mlp_ps", bufs=3, space="PSUM") as mlp_ps,
            tc.tile_pool(name="scat_ps", bufs=5, space="PSUM") as scat_ps,
            tc.tile_pool(name="hpool", bufs=6) as hpool,
            tc.tile_pool(name="spool", bufs=6) as spool,
            tc.tile_pool(name="opool", bufs=2) as opool,
        ):
            # ---- constants / inputs to SBUF
            w_sb = cpool.tile([C + 1, 2 * C], BF16)
            nc.sync.dma_start(w_sb[:], w.ap())
            pos_sb = cpool.tile([128, NPAIR], BF16)
            nc.sync.dma_start(pos_sb[:], pos.ap())
            iota_sb = cpool.tile([128, KB * NCH], BF16)
            nc.gpsimd.iota(
                iota_sb[:], pattern=[[0, KB], [1, NCH]], base=0,
                channel_multiplier=0,
                allow_small_or_imprecise_dtypes=True,
            )

            def _emit_body():
                _emit_pairs(nc, tc, xhi, xlo, xpool, w_sb, pos_sb, iota_sb,
                            out, mlp_ps, scat_ps, hpool, spool, opool)

            if reps == 1:
                _emit_body()
            else:
                with tc.For_i(0, reps, 1):
                    _emit_body()

    nc.compile()
    return nc


def _emit_pairs(nc, tc, xhi, xlo, xpool, w_sb, pos_sb, iota_sb, out,
                mlp_ps, scat_ps, hpool, spool, opool):
    whi = w_sb[:, 0:C]
    wlo = w_sb[:, C:2 * C]
    ob = None
    ps = None
    xhi_seg = xlo_seg = None
    for tb in range(0, NPAIR, KB):
        npr = min(KB, NPAIR - tb)

        # ---- MLP: one PSUM bank holds KB pairs' [128, 64] outputs
        mpw = mlp_ps.tile([128, KB * C], F32)
        for k in range(npr):
            t = tb + k
            if t % SEGP == 0:
                nseg = min(SEGP, NPAIR - t)
                xhi_seg = xpool.tile([C + 1, 128 * SEGP], BF16)
                nc.sync.dma_start(
                    xhi_seg[:, 0:128 * nseg],
                    xhi.ap()[:, t * 128:(t + nseg) * 128],
                )
                xlo_seg = xpool.tile([C + 1, 128 * SEGP], BF16)
                nc.sync.dma_start(
                    xlo_seg[:, 0:128 * nseg],
                    xlo.ap()[:, t * 128:(t + nseg) * 128],
                )
            xh = xhi_seg[:, bass.ts(t % SEGP, 128)]
            xl = xlo_seg[:, bass.ts(t % SEGP, 128)]
            mpk = mpw[:, bass.ts(k, C)]
            nc.tensor.matmul(mpk, lhsT=xh, rhs=whi, start=True, stop=False)
            nc.tensor.matmul(mpk, lhsT=xh, rhs=wlo, start=False, stop=False)
            nc.tensor.matmul(mpk, lhsT=xl, rhs=whi, start=False, stop=True)

        # ---- h hi/lo for the whole batch (one ACT + one DVE op)
        htw = hpool.tile([128, KB * 2 * C], BF16)
        ht3 = htw[:].rearrange("p (k d) -> p k d", d=2 * C)
        hi_v = ht3[:, 0:npr, 0:C]
        lo_v = ht3[:, 0:npr, C:2 * C]
        mp_v = mpw[:, 0:npr * C].rearrange("p (k c) -> p k c", c=C)
        nc.scalar.activation(hi_v, mp_v, mybir.ActivationFunctionType.Relu)
        nc.vector.scalar_tensor_tensor(
            lo_v, mp_v, 0.0, hi_v,
            op0=mybir.AluOpType.max, op1=mybir.AluOpType.subtract,
        )

        # ---- one-hot S for the whole batch (one DVE op)
        stw = spool.tile([128, KB * NCH], BF16)
        st3 = stw[:, 0:npr * NCH].rearrange("p (k n) -> p k n", n=NCH)
        io3 = iota_sb[:, 0:npr * NCH].rearrange("p (k n) -> p k n", n=NCH)
        pos_v = pos_sb[:, tb:tb + npr].to_broadcast([128, npr, NCH])
        nc.vector.tensor_tensor(
            st3, io3, pos_v, op=mybir.AluOpType.is_equal,
        )

        # ---- scatter matmuls + PSUM->SBUF copies + grouped DMA out
        for k in range(npr):
            t = tb + k
            if t % 2 == 0:
                ps = scat_ps.tile([128, 2 * NCH], F32)
                po = 0
            else:
                po = NCH
            hta = ht3[:, k, :]
            stk = stw[:, bass.ts(k, NCH)]
            nc.tensor.matmul(
                ps[0:64, po:po + NCH], lhsT=hta[0:64, 0:C],
                rhs=stk[0:64, :],
                start=True, stop=False, tile_position=(0, 0),
            )
            nc.tensor.matmul(
                ps[0:64, po:po + NCH], lhsT=hta[0:64, C:2 * C],
                rhs=stk[0:64, :],
                start=False, stop=True, tile_position=(0, 0),
            )
            nc.tensor.matmul(
                ps[64:128, po:po + NCH], lhsT=hta[64:128, 0:C],
                rhs=stk[64:128, :],
                start=True, stop=False, tile_position=(64, 64),
            )
            nc.tensor.matmul(
                ps[64:128, po:po + NCH], lhsT=hta[64:128, C:2 * C],
                rhs=stk[64:128, :],
                start=False, stop=True, tile_position=(64, 64),
            )

            j = t % GRP
            if j == 0:
                ob = opool.tile([128, GRP * NCH], F32)
            if t % 2 == 1:
                dst = ob[:, (j - 1) * NCH:(j + 1) * NCH]
                if (t // 2) % 2 == 0:
                    nc.vector.tensor_copy(dst, ps[:])
                else:
                    nc.scalar.activation(
                        dst, ps[:], mybir.ActivationFunctionType.Copy,
                    )
            elif t == NPAIR - 1:
                nc.vector.tensor_copy(ob[:, j * NCH:(j + 1) * NCH],
                                      ps[:, 0:NCH])

            if t == NPAIR - 1:
                dst = out.ap()[:, NCHUNK * NCH - NCH:]
                nc.sync.dma_start(dst, ob[0:64, j * NCH:(j + 1) * NCH])
            elif j == GRP - 1:
                g = t // GRP
                span = out.ap()[:, g * GRP * 2 * NCH:(g + 1) * GRP * 2 * NCH]
                span = span.rearrange("c (j x) -> c j x", j=GRP)
                nc.sync.dma_start(span[:, :, 0:NCH], ob[0:64, :])
                nc.sync.dma_start(span[:, :, NCH:2 * NCH], ob[64:128, :])


class _Runner:
    """Compile-once executor for the SPMD program on 8 cores."""

    def __init__(self, nc):
        self.nc = nc
        bass2jax.install_neuronx_cc_hook()
        part_name = (nc.partition_id_tensor.name
                     if nc.partition_id_tensor else None)
        in_names, out_names, out_avals = [], [], []
        for alloc in nc.m.functions[0].allocations:
            if not isinstance(alloc, mybir.MemoryLocationSet):
                continue
            name = alloc.memorylocations[0].name
            if alloc.kind == "ExternalInput":
                if name != part_name:
                    in_names.append(name)
            elif alloc.kind == "ExternalOutput":
                out_names.append(name)
                out_avals.append(jax.core.ShapedArray(
                    tuple(alloc.tensor_shape), mybir.dt.np(alloc.dtype)))
        self.in_names = in_names
        self.out_names = out_names
        self.out_avals = out_avals
        n_io = len(in_names) + len(out_names)

        devices = jax.devices()[:B]
        self.mesh = Mesh(np.asarray(devices), ("core",))
        all_in_names = list(in_names) + list(out_names)
        if part_name is not None:
            all_in_names.append(part_name)
        all_in_names = tuple(all_in_names)

        def _body(*args):
            operands = list(args)
            if part_name is not None:
                operands.append(bass2jax.partition_id_tensor())
            outs = bass2jax._bass_exec_p.bind(
                *operands,
                out_avals=tuple(out_avals),
                in_names=all_in_names,
                out_names=tuple(out_names),
                lowering_input_output_aliases=(),
                sim_require_finite=True,
                sim_require_nnan=True,
                nc=nc,
            )
            return tuple(outs)

        self.fn = jax.jit(
            shard_map(
                _body, mesh=self.mesh,
                in_specs=(PartitionSpec("core"),) * n_io,
                out_specs=(PartitionSpec("core"),) * len(out_names),
                check_rep=False,
            ),
            keep_unused=True,
        )
        # persistent pre-zeroed "output seed" buffers (kernel writes every
        # element, so their contents are never observed). Created on-device
        # to avoid shipping hundreds of MB through the axon tunnel.
        from jax.sharding import NamedSharding
        import jax.numpy as jnp
        sh = NamedSharding(self.mesh, PartitionSpec("core"))
        self.zero_outs = [
            jax.jit(
                lambda a=a: jnp.zeros((B * a.shape[0], *a.shape[1:]), a.dtype),
                out_shardings=sh,
            )()
            for a in out_avals
        ]

    def to_device(self, arr):
        from jax.sharding import NamedSharding
        return jax.device_put(arr, NamedSharding(self.mesh, PartitionSpec("core")))

    def concat_inputs(self, in_maps):
        return [
            np.concatenate([m[name] for m in in_maps], axis=0)
            for name in self.in_names
        ]

    def run(self, dev_or_np_inputs):
        return self.fn(*dev_or_np_inputs, *self.zero_outs)

    def run_maps(self, in_maps, fetch=True):
        outs = self.run(self.concat_inputs(in_maps))
        if not fetch:
            jax.block_until_ready(outs)
            return None
        res = []
        for c in range(B):
            res.append({
                name: np.asarray(outs[i]).reshape(
                    B, *self.out_avals[i].shape)[c]
                for i, name in enumerate(self.out_names)
            })
        return res


def _get_runner(reps=1):
    key = f"runner{reps}"
    if key not in _cache:
        _cache[key] = _Runner(_build_program(reps))
    return _cache[key]


def _split_hi_lo(a):
    hi = a.astype(NP_BF16)
    lo = (a - hi.astype(np.float32)).astype(NP_BF16)
    return hi, lo


def _host_prep(pillar_features, voxel_coords, topk_w, topk_b, bn_gamma,
               bn_beta, bn_mean, bn_var):
    """Build per-core input maps."""
    s = (bn_gamma / np.sqrt(bn_var + EPS)).astype(np.float32)
    wf = (topk_w * s[None, :]).astype(np.float32)
    bf = ((topk_b - bn_mean) * s + bn_beta).astype(np.float32)
    w_aug = np.concatenate([wf, bf[None, :]], axis=0)  # [65, 64] f32
    whi, wlo = _split_hi_lo(w_aug)
    w_in = np.concatenate([whi, wlo], axis=1)  # [65, 128] bf16

    in_maps = []
    for b in range(B):
        sl = slice(b * P_PER, (b + 1) * P_PER)
        cb = voxel_coords[sl]
        xb = pillar_features[sl]
        valid = cb[:, 4] != -1
        g = (cb[valid, 1] + cb[valid, 2] * NX + cb[valid, 3]).astype(np.int64)
        xv = xb[valid]
        order = np.argsort(g, kind="stable")
        g = g[order]
        xv = xv[order]

        bounds = np.searchsorted(g, np.arange(NCHUNK + 1) * NCH)
        ncs = np.diff(bounds)
        if ncs.max() > RPC:
            raise RuntimeError(
                f"chunk with {ncs.max()} pillars exceeds capacity {RPC}"
            )

        # padded layout: row 64*c + j = j-th pillar of chunk c
        rows = np.arange(NCHUNK) * RPC  # start row of each chunk
        src = np.full(NROW, -1, np.int64)
        posm = np.full((128, NPAIR), -1.0, np.float32)
        for c in range(NCHUNK):
            lo_, hi_ = bounds[c], bounds[c + 1]
            n = hi_ - lo_
            if n == 0:
                continue
            r0 = rows[c]
            src[r0:r0 + n] = np.arange(lo_, hi_)
            t = c // 2
            p0 = (c % 2) * RPC
            posm[p0:p0 + n, t] = (g[lo_:hi_] - c * NCH).astype(np.float32)

        xpad = np.zeros((NROW, C + 1), np.float32)
        sel = src >= 0
        xpad[sel, :C] = xv[src[sel]]
        xpad[:, C] = 1.0
        xhi, xlo = _split_hi_lo(xpad.T)  # [65, NROW] bf16 each

        in_maps.append({
            "xhi": np.ascontiguousarray(xhi),
            "xlo": np.ascontiguousarray(xlo),
            "w": w_in,
            "pos": posm.astype(NP_BF16),
        })
    return in_maps


def kernel(**inputs):
    pillar_features = np.asarray(inputs["pillar_features"], np.float32)
    voxel_coords = np.asarray(inputs["voxel_coords"])
    topk_w = np.asarray(inputs["topk_w"], np.float32)
    topk_b = np.asarray(inputs["topk_b"], np.float32)
    bn_gamma = np.asarray(inputs["bn_gamma"], np.float32)
    bn_beta = np.asarray(inputs["bn_beta"], np.float32)
    bn_mean = np.asarray(inputs["bn_mean"], np.float32)
    bn_var = np.asarray(inputs["bn_var"], np.float32)
    assert int(np.asarray(inputs["batch_size"])) == B
    assert int(np.asarray(inputs["nx"])) == NX
    assert int(np.asarray(inputs["ny"])) == NY

    in_maps = _host_prep(pillar_features, voxel_coords, topk_w, topk_b,
                         bn_gamma, bn_beta, bn_mean, bn_var)
    res = _get_runner().run_maps(in_maps)
    out = np.stack([res[b]["out"] for b in range(B)], axis=0)
    return out.reshape(B, C, NY, NX)
